# revision 6
# baseline (speedup 1.0000x reference)
"""VQ codebook pairwise squared-euclidean distances on 8 trn2 NeuronCores.

out[n, u] = ||x_n||^2 + ||w_u||^2 - 2 * x_n . w_u
  inputs: [16384, 1024] f32, w: [4096, 1024] f32 -> out [16384, 4096] f32

Strategy: data-parallel shard of N across 8 cores (2048 rows each), W
replicated. Per core: bf16 GEMM on the tensor engine (fp32 PSUM accum),
epilogue fuses the two rank-1 terms on ScalarE (per-partition bias
-2*psum + x_sq) and VectorE (+w_sq broadcast tile). Host preps bf16
transposed layouts (K-major) so no on-device transposes are needed.
"""

import sys

import ml_dtypes
import numpy as np

if "/opt/trn_rl_repo" not in sys.path:
    sys.path.insert(0, "/opt/trn_rl_repo")

N, D, U = 16384, 1024, 4096
NCORES = 8
NS = N // NCORES  # 2048 rows per core
P = 128
KT = D // P  # 8 k-tiles
MT = NS // P  # 16 m-tiles per core
UT = U // 512  # 8 u-tiles of 512 cols
MC = 4  # m-tiles per xt load chunk (512 cols)

_cache = {}


def _build():
    import concourse.bacc as bacc
    import concourse.mybir as mybir
    import concourse.tile as tile

    dt = mybir.dt
    AF = mybir.ActivationFunctionType
    ALU = mybir.AluOpType

    nc = bacc.Bacc("TRN2", debug=False, target_bir_lowering=False)
    # Inputs are host-pre-tiled: block b holds [p=128, k=8, c=512] with the
    # SBUF-resident layout, so every load is one fully contiguous 1 MB DMA.
    xt_d = nc.dram_tensor("xt", [MT // MC, P, KT, 512], dt.bfloat16, kind="ExternalInput")
    wt_d = nc.dram_tensor("wt", [UT, P, KT, 512], dt.bfloat16, kind="ExternalInput")
    xsq_d = nc.dram_tensor("xsq", [P, MT], dt.float32, kind="ExternalInput")
    wsq_d = nc.dram_tensor("wsq", [P, U], dt.float32, kind="ExternalInput")
    out_d = nc.dram_tensor("out", [NS, U], dt.float32, kind="ExternalOutput")

    with tile.TileContext(nc) as tc:
        with (
            tc.tile_pool(name="const", bufs=1) as cpool,
            tc.tile_pool(name="psum", bufs=4, space="PSUM") as psum_pool,
            tc.tile_pool(name="outp", bufs=8) as out_pool,
        ):
            # Small epilogue constants ride the scalar HWDGE ring, which is
            # otherwise idle until outputs start.
            xsq_sb = cpool.tile([P, MT], dt.float32, tag="xsq")
            nc.scalar.dma_start(xsq_sb[:], xsq_d[:, :])
            wsq_sb = cpool.tile([P, U], dt.float32, tag="wsq")
            nc.scalar.dma_start(wsq_sb[:], wsq_d[:, :])

            xt_sb = {}
            wt_sb = {}

            def load_xt(mc):
                t = cpool.tile([P, KT, 512], dt.bfloat16, tag=f"xt_{mc}")
                nc.sync.dma_start(t[:], xt_d[mc])
                xt_sb[mc] = t

            def load_wt(u):
                t = cpool.tile([P, KT, 512], dt.bfloat16, tag=f"wt_{u}")
                nc.sync.dma_start(t[:], wt_d[u])
                wt_sb[u] = t

            # DMA program order = consumption priority on the sync ring.
            load_wt(0)
            load_xt(0)
            load_xt(1)
            load_xt(2)
            load_xt(3)
            for u in range(1, UT):
                load_wt(u)

            for u in range(UT):
                for m in range(MT):
                    mc, mo = divmod(m, MC)
                    ps = psum_pool.tile([P, 512], dt.float32, tag="ps")
                    for k in range(KT):
                        nc.tensor.matmul(
                            ps[:],
                            xt_sb[mc][:, k, mo * P : (mo + 1) * P],
                            wt_sb[u][:, k, :],
                            start=(k == 0),
                            stop=(k == KT - 1),
                        )
                    ot = out_pool.tile([P, 512], dt.float32, tag="ot")
                    nc.scalar.activation(
                        ot[:], ps[:], AF.Identity, bias=xsq_sb[:, m : m + 1], scale=-2.0
                    )
                    nc.vector.tensor_tensor(
                        ot[:], ot[:], wsq_sb[:, u * 512 : (u + 1) * 512], ALU.add
                    )
                    # Outputs go out on the scalar HWDGE ring so they don't
                    # FIFO behind the input stream on the sync ring.
                    nc.scalar.dma_start(
                        out_d[m * P : (m + 1) * P, u * 512 : (u + 1) * 512], ot[:]
                    )
    nc.compile()
    return nc


def _get_nc():
    if "nc" not in _cache:
        _cache["nc"] = _build()
    return _cache["nc"]


def _prep_inputs(inputs, w):
    bf16 = ml_dtypes.bfloat16
    x = np.ascontiguousarray(np.asarray(inputs, dtype=np.float32))
    wf = np.ascontiguousarray(np.asarray(w, dtype=np.float32))

    # [u, p, k, c]: element = w[row = u*512 + c, d = k*128 + p]
    wt = np.ascontiguousarray(
        wf.astype(bf16).reshape(UT, 512, KT, P).transpose(0, 3, 2, 1)
    )
    w_sq = (wf.astype(np.float64) ** 2).sum(-1).astype(np.float32)  # [U]
    wsq_bc = np.ascontiguousarray(np.broadcast_to(w_sq[None, :], (P, U)))
    x_sq = (x.astype(np.float64) ** 2).sum(-1).astype(np.float32)  # [N]

    in_maps = []
    for c in range(NCORES):
        xs = x[c * NS : (c + 1) * NS]
        # [mc, p, k, c]: element = x[n = mc*512 + col, d = k*128 + p]
        xt = np.ascontiguousarray(
            xs.astype(bf16).reshape(MT // MC, 512, KT, P).transpose(0, 3, 2, 1)
        )
        xsq_t = np.ascontiguousarray(
            x_sq[c * NS : (c + 1) * NS].reshape(MT, P).T
        )  # [P, MT]
        in_maps.append({"xt": xt, "wt": wt, "xsq": xsq_t, "wsq": wsq_bc})
    return in_maps


def run(inputs, w, trace=False, **trace_kwargs):
    """Run on hardware; returns (out, BassKernelResults)."""
    from concourse.bass_utils import run_bass_kernel_spmd

    nc = _get_nc()
    in_maps = _prep_inputs(inputs, w)
    res = run_bass_kernel_spmd(
        nc, in_maps, list(range(NCORES)), trace=trace, **trace_kwargs
    )
    out = np.concatenate([r["out"] for r in res.results], axis=0)
    return np.ascontiguousarray(out, dtype=np.float32), res


def kernel(inputs, w):
    out, _ = run(inputs, w)
    return out


# revision 7
# speedup vs baseline: 1.7412x; 1.7412x over previous
"""VQ codebook pairwise squared-euclidean distances on 8 trn2 NeuronCores.

out[n, u] = ||x_n||^2 + ||w_u||^2 - 2 * x_n . w_u
  inputs: [16384, 1024] f32, w: [4096, 1024] f32 -> out [16384, 4096] f32

Strategy: data-parallel shard of N across 8 cores (2048 rows each), W
replicated. Per core: fp8(e4m3) GEMM on the tensor engine in DoubleRow
perf mode (2 MACs/cell/cycle, fp32 PSUM accum; w is pre-scaled by 64 to
stay in e4m3's normal range, undone in the epilogue scale). The two
rank-1 terms are fused in the epilogue on ScalarE (per-partition bias:
-2/64*psum + x_sq) and VectorE (+w_sq broadcast tile). Host preps fp8
K-packed layouts so every input load is one contiguous 1 MB DMA and no
on-device transposes are needed. Output stores alternate between the
two HWDGE rings (sync/scalar) to sustain the fp8-rate store stream.
"""

import sys

import ml_dtypes
import numpy as np

if "/opt/trn_rl_repo" not in sys.path:
    sys.path.insert(0, "/opt/trn_rl_repo")

N, D, U = 16384, 1024, 4096
NCORES = 8
NS = N // NCORES  # 2048 rows per core
P = 128
KK = D // 256  # 4 DoubleRow super k-tiles (256 contraction each)
MT = NS // P  # 16 m-tiles per core
UT = U // 512  # 8 u-tiles of 512 cols
MC = 4  # m-tiles per xt load chunk (512 cols)
WSCALE = 64.0  # w pre-scale into e4m3 normal range (power of 2: exact)

_cache = {}


def _build():
    import concourse.bacc as bacc
    import concourse.mybir as mybir
    import concourse.tile as tile

    dt = mybir.dt
    AF = mybir.ActivationFunctionType
    ALU = mybir.AluOpType
    DR = mybir.MatmulPerfMode.DoubleRow

    nc = bacc.Bacc("TRN2", debug=False, target_bir_lowering=False)
    # Host-pre-packed fp8 inputs: block b holds [p=128, kk, i, c] where the
    # contraction index is d = kk*256 + i*128 + p (DoubleRow packs pairs
    # (p, i) into one PE cell). Each block is one contiguous 1 MB DMA.
    xt_d = nc.dram_tensor("xt", [MT // MC, P, KK, 2, 512], dt.float8e4, kind="ExternalInput")
    wt_d = nc.dram_tensor("wt", [UT, P, KK, 2, 512], dt.float8e4, kind="ExternalInput")
    xsq_d = nc.dram_tensor("xsq", [P, MT], dt.float32, kind="ExternalInput")
    wsq_d = nc.dram_tensor("wsq", [P, U], dt.float32, kind="ExternalInput")
    out_d = nc.dram_tensor("out", [NS, U], dt.float32, kind="ExternalOutput")

    with tile.TileContext(nc) as tc:
        with (
            tc.tile_pool(name="const", bufs=1) as cpool,
            tc.tile_pool(name="psum", bufs=4, space="PSUM") as psum_pool,
            tc.tile_pool(name="outp", bufs=8) as out_pool,
        ):
            # Small epilogue constants ride the scalar HWDGE ring, which is
            # otherwise idle until outputs start.
            xsq_sb = cpool.tile([P, MT], dt.float32, tag="xsq")
            nc.scalar.dma_start(xsq_sb[:], xsq_d[:, :])
            wsq_sb = cpool.tile([P, U], dt.float32, tag="wsq")
            nc.scalar.dma_start(wsq_sb[:], wsq_d[:, :])

            xt_sb = {}
            wt_sb = {}

            def load_xt(mc):
                t = cpool.tile([P, KK, 2, 512], dt.float8e4, tag=f"xt_{mc}")
                nc.sync.dma_start(t[:], xt_d[mc])
                xt_sb[mc] = t

            def load_wt(u):
                t = cpool.tile([P, KK, 2, 512], dt.float8e4, tag=f"wt_{u}")
                nc.sync.dma_start(t[:], wt_d[u])
                wt_sb[u] = t

            # DMA program order = consumption priority on the sync ring.
            load_wt(0)
            load_xt(0)
            load_xt(1)
            load_xt(2)
            load_xt(3)
            for u in range(1, UT):
                load_wt(u)

            for u in range(UT):
                for m in range(MT):
                    mc, mo = divmod(m, MC)
                    ps = psum_pool.tile([P, 512], dt.float32, tag="ps")
                    for kk in range(KK):
                        nc.tensor.matmul(
                            ps[:],
                            xt_sb[mc][:, kk, :, mo * P : (mo + 1) * P],
                            wt_sb[u][:, kk, :, :],
                            start=(kk == 0),
                            stop=(kk == KK - 1),
                            perf_mode=DR,
                        )
                    ot = out_pool.tile([P, 512], dt.float32, tag="ot")
                    nc.scalar.activation(
                        ot[:],
                        ps[:],
                        AF.Identity,
                        bias=xsq_sb[:, m : m + 1],
                        scale=-2.0 / WSCALE,
                    )
                    nc.vector.tensor_tensor(
                        ot[:], ot[:], wsq_sb[:, u * 512 : (u + 1) * 512], ALU.add
                    )
                    # Alternate output stores across the two HWDGE rings so
                    # the fp8-rate store stream isn't ring-limited.
                    eng = nc.sync if (u * MT + m) % 2 == 0 else nc.scalar
                    eng.dma_start(
                        out_d[m * P : (m + 1) * P, u * 512 : (u + 1) * 512], ot[:]
                    )
    nc.compile()
    return nc


def _get_nc():
    if "nc" not in _cache:
        _cache["nc"] = _build()
    return _cache["nc"]


def _prep_inputs(inputs, w):
    f8 = ml_dtypes.float8_e4m3
    x = np.ascontiguousarray(np.asarray(inputs, dtype=np.float32))
    wf = np.ascontiguousarray(np.asarray(w, dtype=np.float32))

    # [u, p, kk, i, c]: element = w[u*512 + c, kk*256 + i*128 + p] * WSCALE
    wt = np.ascontiguousarray(
        (wf * WSCALE).astype(f8).reshape(UT, 512, KK, 2, P).transpose(0, 4, 2, 3, 1)
    )
    w_sq = (wf.astype(np.float64) ** 2).sum(-1).astype(np.float32)  # [U]
    wsq_bc = np.ascontiguousarray(np.broadcast_to(w_sq[None, :], (P, U)))
    x_sq = (x.astype(np.float64) ** 2).sum(-1).astype(np.float32)  # [N]

    in_maps = []
    for c in range(NCORES):
        xs = x[c * NS : (c + 1) * NS]
        # [mc, p, kk, i, c]: element = x[n = mc*512 + col, d = kk*256 + i*128 + p]
        xt = np.ascontiguousarray(
            xs.astype(f8).reshape(MT // MC, 512, KK, 2, P).transpose(0, 4, 2, 3, 1)
        )
        xsq_t = np.ascontiguousarray(
            x_sq[c * NS : (c + 1) * NS].reshape(MT, P).T
        )  # [P, MT]
        in_maps.append({"xt": xt, "wt": wt, "xsq": xsq_t, "wsq": wsq_bc})
    return in_maps


def run(inputs, w, trace=False, **trace_kwargs):
    """Run on hardware; returns (out, BassKernelResults)."""
    from concourse.bass_utils import run_bass_kernel_spmd

    nc = _get_nc()
    in_maps = _prep_inputs(inputs, w)
    res = run_bass_kernel_spmd(
        nc, in_maps, list(range(NCORES)), trace=trace, **trace_kwargs
    )
    out = np.concatenate([r["out"] for r in res.results], axis=0)
    return np.ascontiguousarray(out, dtype=np.float32), res


def kernel(inputs, w):
    out, _ = run(inputs, w)
    return out


# revision 11
# speedup vs baseline: 1.8851x; 1.0827x over previous
"""VQ codebook pairwise squared-euclidean distances on 8 trn2 NeuronCores.

out[n, u] = ||x_n||^2 + ||w_u||^2 - 2 * x_n . w_u
  inputs: [16384, 1024] f32, w: [4096, 1024] f32 -> out [16384, 4096] f32

Strategy: data-parallel shard of N across 8 cores (2048 rows each), W
replicated. Per core: fp8(e4m3) GEMM on the tensor engine in DoubleRow
perf mode (2 MACs/cell/cycle, fp32 PSUM accum; w is pre-scaled by 64 to
stay in e4m3's normal range, undone in the epilogue scale). The two
rank-1 terms are fused in the epilogue on ScalarE (per-partition bias:
-2/64*psum + x_sq) and VectorE (+w_sq broadcast tile). Host preps fp8
K-packed layouts so every input load is one contiguous 1 MB DMA and no
on-device transposes are needed. Output stores alternate between the
two HWDGE rings (sync/scalar) to sustain the fp8-rate store stream.
"""

import sys

import ml_dtypes
import numpy as np

if "/opt/trn_rl_repo" not in sys.path:
    sys.path.insert(0, "/opt/trn_rl_repo")

N, D, U = 16384, 1024, 4096
NCORES = 8
NS = N // NCORES  # 2048 rows per core
P = 128
KK = D // 256  # 4 DoubleRow super k-tiles (256 contraction each)
MT = NS // P  # 16 m-tiles per core
UT = U // 512  # 8 u-tiles of 512 cols
MC = 4  # m-tiles per xt load chunk (512 cols)
WSCALE = 64.0  # w pre-scale into e4m3 normal range (power of 2: exact)

_cache = {}


def _build():
    import concourse.bacc as bacc
    import concourse.mybir as mybir
    import concourse.tile as tile

    dt = mybir.dt
    AF = mybir.ActivationFunctionType
    ALU = mybir.AluOpType
    DR = mybir.MatmulPerfMode.DoubleRow

    nc = bacc.Bacc("TRN2", debug=False, target_bir_lowering=False)
    # Host-pre-packed fp8 inputs: block b holds [p=128, kk, i, c] where the
    # contraction index is d = kk*256 + i*128 + p (DoubleRow packs pairs
    # (p, i) into one PE cell). Each block is one contiguous 1 MB DMA.
    xt_d = nc.dram_tensor("xt", [MT // MC, P, KK, 2, 512], dt.float8e4, kind="ExternalInput")
    wt_d = nc.dram_tensor("wt", [UT, P, KK, 2, 512], dt.float8e4, kind="ExternalInput")
    xsq_d = nc.dram_tensor("xsq", [P, MT], dt.float32, kind="ExternalInput")
    wsq_d = nc.dram_tensor("wsq", [P, U], dt.bfloat16, kind="ExternalInput")
    out_d = nc.dram_tensor("out", [NS, U], dt.float32, kind="ExternalOutput")

    with tile.TileContext(nc) as tc:
        with (
            tc.tile_pool(name="const", bufs=1) as cpool,
            tc.tile_pool(name="psum", bufs=4, space="PSUM") as psum_pool,
            tc.tile_pool(name="outp", bufs=24) as out_pool,
        ):
            # Small epilogue constants ride the scalar HWDGE ring, which is
            # otherwise idle until outputs start.
            xsq_sb = cpool.tile([P, MT], dt.float32, tag="xsq")
            nc.scalar.dma_start(xsq_sb[:], xsq_d[:, :])
            wsq_sb = cpool.tile([P, U], dt.bfloat16, tag="wsq")
            nc.scalar.dma_start(wsq_sb[:], wsq_d[:, :])

            xt_sb = {}
            wt_sb = {}

            def load_xt(mc):
                t = cpool.tile([P, KK, 2, 512], dt.float8e4, tag=f"xt_{mc}")
                nc.sync.dma_start(t[:], xt_d[mc])
                xt_sb[mc] = t

            def load_wt(u):
                t = cpool.tile([P, KK, 2, 512], dt.float8e4, tag=f"wt_{u}")
                nc.sync.dma_start(t[:], wt_d[u])
                wt_sb[u] = t

            # DMA program order = consumption priority on the sync ring.
            load_wt(0)
            load_xt(0)
            load_xt(1)
            load_xt(2)
            load_xt(3)
            for u in range(1, UT):
                load_wt(u)

            for u in range(UT):
                for m in range(MT):
                    mc, mo = divmod(m, MC)
                    ps = psum_pool.tile([P, 512], dt.float32, tag="ps")
                    for kk in range(KK):
                        nc.tensor.matmul(
                            ps[:],
                            xt_sb[mc][:, kk, :, mo * P : (mo + 1) * P],
                            wt_sb[u][:, kk, :, :],
                            start=(kk == 0),
                            stop=(kk == KK - 1),
                            perf_mode=DR,
                        )
                    ot = out_pool.tile([P, 512], dt.float32, tag="ot")
                    nc.scalar.activation(
                        ot[:],
                        ps[:],
                        AF.Identity,
                        bias=xsq_sb[:, m : m + 1],
                        scale=-2.0 / WSCALE,
                    )
                    nc.vector.tensor_tensor(
                        ot[:], ot[:], wsq_sb[:, u * 512 : (u + 1) * 512], ALU.add
                    )
                    # Alternate output stores across the two HWDGE rings so
                    # the fp8-rate store stream isn't ring-limited.
                    eng = nc.sync if (u * MT + m) % 2 == 0 else nc.scalar
                    eng.dma_start(
                        out_d[m * P : (m + 1) * P, u * 512 : (u + 1) * 512], ot[:]
                    )
    nc.compile()
    return nc


def _get_nc():
    if "nc" not in _cache:
        _cache["nc"] = _build()
    return _cache["nc"]


def _prep_inputs(inputs, w):
    f8 = ml_dtypes.float8_e4m3
    x = np.ascontiguousarray(np.asarray(inputs, dtype=np.float32))
    wf = np.ascontiguousarray(np.asarray(w, dtype=np.float32))

    # [u, p, kk, i, c]: element = w[u*512 + c, kk*256 + i*128 + p] * WSCALE
    wt = np.ascontiguousarray(
        (wf * WSCALE).astype(f8).reshape(UT, 512, KK, 2, P).transpose(0, 4, 2, 3, 1)
    )
    w_sq = (wf.astype(np.float64) ** 2).sum(-1).astype(ml_dtypes.bfloat16)  # [U]
    wsq_bc = np.ascontiguousarray(np.broadcast_to(w_sq[None, :], (P, U)))
    x_sq = (x.astype(np.float64) ** 2).sum(-1).astype(np.float32)  # [N]

    in_maps = []
    for c in range(NCORES):
        xs = x[c * NS : (c + 1) * NS]
        # [mc, p, kk, i, c]: element = x[n = mc*512 + col, d = kk*256 + i*128 + p]
        xt = np.ascontiguousarray(
            xs.astype(f8).reshape(MT // MC, 512, KK, 2, P).transpose(0, 4, 2, 3, 1)
        )
        xsq_t = np.ascontiguousarray(
            x_sq[c * NS : (c + 1) * NS].reshape(MT, P).T
        )  # [P, MT]
        in_maps.append({"xt": xt, "wt": wt, "xsq": xsq_t, "wsq": wsq_bc})
    return in_maps


def run(inputs, w, trace=False, **trace_kwargs):
    """Run on hardware; returns (out, BassKernelResults)."""
    from concourse.bass_utils import run_bass_kernel_spmd

    nc = _get_nc()
    in_maps = _prep_inputs(inputs, w)
    res = run_bass_kernel_spmd(
        nc, in_maps, list(range(NCORES)), trace=trace, **trace_kwargs
    )
    out = np.concatenate([r["out"] for r in res.results], axis=0)
    return np.ascontiguousarray(out, dtype=np.float32), res


def kernel(inputs, w):
    out, _ = run(inputs, w)
    return out
